# revision 9
# baseline (speedup 1.0000x reference)
"""Trainium2 Bass kernel for FastMaskedDense1D.update_site (index=300 regime).

Math (reference semantics, EXCLUSIVE=1):
    cache[:, index-1, :] = inputs                      (scatter)
    cache_i = cache[:, :index+1, :].reshape(B, -1)
    kernel_i = kernel.reshape(S, IF, S, F)[:index+1, :, index, :]
    kernel_i *= (arange(index+1) <= index-1)[:, None, None]   (mask)
    y = cache_i @ kernel_i.reshape(-1, F) + bias[index]

Because the mask zeroes site `index`, only sites 0..index-1 contribute, with
site index-1 replaced by `inputs`. So the whole op is one skinny matmul:
    y = A @ Keff + bias,  A: (B, index*IF), Keff: (index*IF, F)

Strategy: data-parallel over the batch across 8 NeuronCores. The host folds
scatter + mask + kernel-slice + bias (ones-column trick) into a per-core
dense problem laid out contraction-major (A^T, contiguous) so each core runs
a single streaming matmul  out^T = Keff^T @ A^T  at full DMA rate.

Precision: the 2e-2 rel-err budget is enormous relative to fp16 rounding
(~4e-4 measured for a 4801-length dot of N(0,1) data), so both A and Keff
ship as plain fp16 — HALF the HBM bytes of fp32 (or of an fp16 hi/lo
pair). fp8 would fail the gate (~3.6e-2) and the PE has no int8, so 2
bytes/element is the true traffic floor (~10.0 MB/core). PSUM accumulates
in fp32 across the 38 k-tiles; one copy casts y^T to fp16 in SBUF at
partition base 0 (walrus-safe), and one 32KB DMA ships it out.

Engine split: the A stream issues from the SP HWDGE queues; kw/tail/out
DMAs ride the Activation HWDGE queues so the main stream never sees
head-of-line blocking at rep/kernel boundaries. PE busy (~9-10us of
512-wide fp16 matmuls) stays under the DMA stream time, keeping the
kernel memory-bound at the HBM roofline (~500-900 GB/s/core depending on
chip contention).

DRAM layout per core:
  at2 (full*128, 1024) fp16 : k-tile-major rows (plain row-major A^T)
  atl (65, 1024) fp16       : the 4801 % 128 tail rows
  km2 (128, T*F) fp16       : per-tile stationary blocks of Keff^T
  out (F, 1024) fp16        : y^T; host transposes + casts back
"""

import numpy as np

BATCH = 8192
SIZE = 512
FEATURES = 16
IN_FEATURES = 16
EXCLUSIVE = 1
NCORES = 8
P = 128
G = 4  # contraction k-tiles per DMA
SPLIT_B = 1  # DMAs per supertile along the batch dim
BUFS = 4

_NC_CACHE: dict = {}


def _build(K_len: int, B: int, F: int, repeats: int = 1):
    """out(F, B) = Keff.T @ A^T via fp16 matmuls.

    K_len need not be a multiple of 128: the first `full` 128-row k-tiles
    stream through the main loop; the remaining r = K_len - full*128 rows
    go through one partial-partition tile (K<128 matmul) so no padded
    bytes ever cross HBM."""
    import concourse.bacc as bacc
    import concourse.mybir as mybir
    from concourse.tile import TileContext

    F32 = mybir.dt.float32
    FP16 = mybir.dt.float16
    full, r = divmod(K_len, P)
    T = full + (1 if r else 0)  # stationary blocks in km2
    assert B % 512 == 0
    NBH = B // 512

    nc = bacc.Bacc("TRN2", target_bir_lowering=False, debug=False)
    AT2 = nc.dram_tensor("at2", (full * P, B), FP16, kind="ExternalInput")
    if r:
        ATL = nc.dram_tensor("atl", (r, B), FP16, kind="ExternalInput")
    KM2 = nc.dram_tensor("km2", (P, T * F), FP16, kind="ExternalInput")
    OUT = nc.dram_tensor("out", (F, B), FP16, kind="ExternalOutput")

    at_view = AT2.ap().rearrange("(q p) b -> p q b", p=P)

    with TileContext(nc) as tc:
        with (
            tc.tile_pool(name="kw", bufs=1) as kwpool,
            tc.tile_pool(name="a", bufs=BUFS) as apool,
            tc.tile_pool(name="o", bufs=2) as opool,
            tc.tile_pool(name="ps", bufs=2, space="PSUM") as pspool,
        ):
            # kw/tail/out DMAs ride the Activation HWDGE queues so the SP
            # queue streams A tiles back-to-back with no head-of-line stalls.
            kw = kwpool.tile([P, T * F], FP16)
            nc.scalar.dma_start(kw[:], KM2.ap())
            for _ in range(repeats):
                ps = [
                    pspool.tile([F, 512], F32, tag=f"ps_{bh}", name=f"ps_{bh}")
                    for bh in range(NBH)
                ]
                t = 0
                first = True
                while t < full:
                    # small first transfer so the PE starts ~3 k-tiles
                    # earlier during pipeline fill; full-size after that
                    g = 1 if (first and full > G) else min(G, full - t)
                    first = False
                    a_tile = apool.tile([P, G, B], FP16, tag="a", name="a_tile")
                    bs = B // SPLIT_B
                    for sb in range(SPLIT_B):
                        nc.sync.dma_start(
                            a_tile[:, :g, sb * bs : (sb + 1) * bs],
                            at_view[:, t : t + g, sb * bs : (sb + 1) * bs],
                        )
                    for gi in range(g):
                        tt = t + gi
                        for bh in range(NBH):
                            bsl = slice(bh * 512, (bh + 1) * 512)
                            nc.tensor.matmul(
                                ps[bh][:],
                                kw[:, tt * F : (tt + 1) * F],
                                a_tile[:, gi, bsl],
                                start=(tt == 0),
                                stop=(tt == full - 1 and not r),
                            )
                    t += g
                if r:
                    a_tail = apool.tile([P, 1, B], FP16, tag="atail", name="a_tail")
                    nc.scalar.dma_start(a_tail[:r, 0, :], ATL.ap())
                    for bh in range(NBH):
                        bsl = slice(bh * 512, (bh + 1) * 512)
                        nc.tensor.matmul(
                            ps[bh][:],
                            kw[:r, full * F : (full + 1) * F],
                            a_tail[:r, 0, bsl],
                            start=(full == 0),
                            stop=True,
                        )
                outsb = opool.tile([F, B], FP16, tag="out", name="outsb")
                for bh in range(NBH):
                    bsl = slice(bh * 512, (bh + 1) * 512)
                    nc.any.tensor_copy(out=outsb[:, bsl], in_=ps[bh][:])
                nc.scalar.dma_start(OUT.ap()[:, :], outsb[:, :])
    nc.compile()
    return nc


def _get_nc(K_len: int, B: int, F: int, repeats: int = 1):
    key = (K_len, B, F, repeats)
    if key not in _NC_CACHE:
        _NC_CACHE[key] = _build(K_len, B, F, repeats)
    return _NC_CACHE[key]


def _prepare(inputs, cache, kernel, bias, index):
    """Host-side fold: returns (in_maps, K_len, B_core, F)."""
    index = int(index)
    B, IF = inputs.shape
    S, F = bias.shape
    assert B % NCORES == 0
    B_core = B // NCORES

    hi_site = index - EXCLUSIVE
    n_sites = hi_site + 1 if hi_site >= 0 else 0
    K_len = n_sites * IF + 1  # +1 = ones column carrying the bias
    full, r = divmod(K_len, P)
    T = full + (1 if r else 0)
    K_pad = T * P  # stationary-side padding only (km2 is tiny)

    # Keff (masked kernel slice) + bias row, zero-padded, fp16, tile-swizzled.
    km = np.zeros((K_pad, F), np.float32)
    if n_sites:
        kr = kernel.reshape(S, IF, S, F)[:n_sites, :, index, :]
        km[: n_sites * IF] = np.asarray(kr, np.float32).reshape(n_sites * IF, F)
    km[n_sites * IF] = np.asarray(bias[index], np.float32)
    khi = km.astype(np.float16)
    KM2 = np.ascontiguousarray(
        khi.reshape(T, P, F).transpose(1, 0, 2).reshape(P, T * F)
    )

    inputs = np.asarray(inputs, np.float32)
    cache = np.asarray(cache, np.float32)
    in_maps = []
    for c in range(NCORES):
        rows = slice(c * B_core, (c + 1) * B_core)
        at = np.zeros((K_len, B_core), np.float32)
        if n_sites:
            at[: n_sites * IF] = (
                cache[rows, :n_sites, :].reshape(B_core, n_sites * IF).T
            )
            at[hi_site * IF : (hi_site + 1) * IF] = inputs[rows].T
        at[n_sites * IF] = 1.0
        ahi = at.astype(np.float16)
        m = {"at2": np.ascontiguousarray(ahi[: full * P]), "km2": KM2}
        if r:
            m["atl"] = np.ascontiguousarray(ahi[full * P :])
        in_maps.append(m)
    return in_maps, K_len, B_core, F


def kernel(inputs, cache, kernel, bias, index):
    from concourse.bass_utils import run_bass_kernel_spmd

    in_maps, K_len, B_core, F = _prepare(inputs, cache, kernel, bias, index)
    nc = _get_nc(K_len, B_core, F)
    res = run_bass_kernel_spmd(nc, in_maps, core_ids=list(range(NCORES)))
    parts = []
    for c in range(NCORES):
        o = np.asarray(res.results[c]["out"])  # (F, B_core) = y^T, fp16
        parts.append(o.T.astype(np.float32))
    return np.ascontiguousarray(np.concatenate(parts, axis=0), dtype=np.float32)


# revision 10
# speedup vs baseline: 1.6865x; 1.6865x over previous
"""Trainium2 Bass kernel for FastMaskedDense1D.update_site (index=300 regime).

Math (reference semantics, EXCLUSIVE=1):
    cache[:, index-1, :] = inputs                      (scatter)
    cache_i = cache[:, :index+1, :].reshape(B, -1)
    kernel_i = kernel.reshape(S, IF, S, F)[:index+1, :, index, :]
    kernel_i *= (arange(index+1) <= index-1)[:, None, None]   (mask)
    y = cache_i @ kernel_i.reshape(-1, F) + bias[index]

Because the mask zeroes site `index`, only sites 0..index-1 contribute, with
site index-1 replaced by `inputs`. So the whole op is one skinny matmul:
    y = A @ Keff + bias,  A: (B, index*IF), Keff: (index*IF, F)

Strategy: data-parallel over the batch across 8 NeuronCores. The host folds
scatter + mask + kernel-slice + bias (ones-column trick) into a per-core
dense problem laid out contraction-major (A^T, contiguous) so each core runs
a single streaming matmul  out^T = Keff^T @ A^T  at full DMA rate.

Precision: the 2e-2 rel-err budget is enormous relative to fp16 rounding
(~4e-4 measured for a 4801-length dot of N(0,1) data), so both A and Keff
ship as plain fp16 — HALF the HBM bytes of fp32 (or of an fp16 hi/lo
pair). fp8 would fail the gate (~3.6e-2) and the PE has no int8, so 2
bytes/element is the true traffic floor (~10.0 MB/core). PSUM accumulates
in fp32 across the 38 k-tiles; one copy casts y^T to fp16 in SBUF at
partition base 0 (walrus-safe), and one 32KB DMA ships it out.

Engine split: the A stream issues from the SP HWDGE queues; kw/tail/out
DMAs ride the Activation HWDGE queues so the main stream never sees
head-of-line blocking at rep/kernel boundaries. PE busy (~9-10us of
512-wide fp16 matmuls) stays under the DMA stream time, keeping the
kernel memory-bound at the HBM roofline (~500-900 GB/s/core depending on
chip contention).

DRAM layout per core:
  at2 (full*128, 1024) fp16 : k-tile-major rows (plain row-major A^T)
  atl (65, 1024) fp16       : the 4801 % 128 tail rows
  km2 (128, T*F) fp16       : per-tile stationary blocks of Keff^T
  out (F, 1024) fp16        : y^T; host transposes + casts back
"""

import numpy as np

BATCH = 8192
SIZE = 512
FEATURES = 16
IN_FEATURES = 16
EXCLUSIVE = 1
NCORES = 8
P = 128
G = 4  # contraction k-tiles per DMA
SPLIT_B = 1  # DMAs per supertile along the batch dim
BUFS = 6  # deep a-tile pipeline: absorbs HBM-contention jitter

_NC_CACHE: dict = {}


def _build(K_len: int, B: int, F: int, repeats: int = 1):
    """out(F, B) = Keff.T @ A^T via fp16 matmuls.

    K_len need not be a multiple of 128: the first `full` 128-row k-tiles
    stream through the main loop; the remaining r = K_len - full*128 rows
    go through one partial-partition tile (K<128 matmul) so no padded
    bytes ever cross HBM."""
    import concourse.bacc as bacc
    import concourse.mybir as mybir
    from concourse.tile import TileContext

    F32 = mybir.dt.float32
    FP16 = mybir.dt.float16
    full, r = divmod(K_len, P)
    T = full + (1 if r else 0)  # stationary blocks in km2
    assert B % 512 == 0
    NBH = B // 512

    nc = bacc.Bacc("TRN2", target_bir_lowering=False, debug=False)
    AT2 = nc.dram_tensor("at2", (full * P, B), FP16, kind="ExternalInput")
    if r:
        ATL = nc.dram_tensor("atl", (r, B), FP16, kind="ExternalInput")
    KM2 = nc.dram_tensor("km2", (P, T * F), FP16, kind="ExternalInput")
    OUT = nc.dram_tensor("out", (F, B), FP16, kind="ExternalOutput")

    at_view = AT2.ap().rearrange("(q p) b -> p q b", p=P)

    with TileContext(nc) as tc:
        with (
            tc.tile_pool(name="kw", bufs=1) as kwpool,
            tc.tile_pool(name="a", bufs=BUFS) as apool,
            tc.tile_pool(name="o", bufs=2) as opool,
            tc.tile_pool(name="ps", bufs=2, space="PSUM") as pspool,
        ):
            # kw/tail/out DMAs ride the Activation HWDGE queues so the SP
            # queue streams A tiles back-to-back with no head-of-line stalls.
            kw = kwpool.tile([P, T * F], FP16)
            nc.scalar.dma_start(kw[:], KM2.ap())
            for _ in range(repeats):
                ps = [
                    pspool.tile([F, 512], F32, tag=f"ps_{bh}", name=f"ps_{bh}")
                    for bh in range(NBH)
                ]
                t = 0
                first = True
                while t < full:
                    # small first transfer so the PE starts ~3 k-tiles
                    # earlier during pipeline fill; full-size after that
                    g = 1 if (first and full > G) else min(G, full - t)
                    first = False
                    a_tile = apool.tile([P, G, B], FP16, tag="a", name="a_tile")
                    bs = B // SPLIT_B
                    for sb in range(SPLIT_B):
                        nc.sync.dma_start(
                            a_tile[:, :g, sb * bs : (sb + 1) * bs],
                            at_view[:, t : t + g, sb * bs : (sb + 1) * bs],
                        )
                    for gi in range(g):
                        tt = t + gi
                        for bh in range(NBH):
                            bsl = slice(bh * 512, (bh + 1) * 512)
                            nc.tensor.matmul(
                                ps[bh][:],
                                kw[:, tt * F : (tt + 1) * F],
                                a_tile[:, gi, bsl],
                                start=(tt == 0),
                                stop=(tt == full - 1 and not r),
                            )
                    t += g
                if r:
                    a_tail = apool.tile([P, 1, B], FP16, tag="atail", name="a_tail")
                    nc.scalar.dma_start(a_tail[:r, 0, :], ATL.ap())
                    for bh in range(NBH):
                        bsl = slice(bh * 512, (bh + 1) * 512)
                        nc.tensor.matmul(
                            ps[bh][:],
                            kw[:r, full * F : (full + 1) * F],
                            a_tail[:r, 0, bsl],
                            start=(full == 0),
                            stop=True,
                        )
                outsb = opool.tile([F, B], FP16, tag="out", name="outsb")
                for bh in range(NBH):
                    bsl = slice(bh * 512, (bh + 1) * 512)
                    nc.any.tensor_copy(out=outsb[:, bsl], in_=ps[bh][:])
                nc.scalar.dma_start(OUT.ap()[:, :], outsb[:, :])
    nc.compile()
    return nc


def _get_nc(K_len: int, B: int, F: int, repeats: int = 1):
    key = (K_len, B, F, repeats)
    if key not in _NC_CACHE:
        _NC_CACHE[key] = _build(K_len, B, F, repeats)
    return _NC_CACHE[key]


def _prepare(inputs, cache, kernel, bias, index):
    """Host-side fold: returns (in_maps, K_len, B_core, F)."""
    index = int(index)
    B, IF = inputs.shape
    S, F = bias.shape
    assert B % NCORES == 0
    B_core = B // NCORES

    hi_site = index - EXCLUSIVE
    n_sites = hi_site + 1 if hi_site >= 0 else 0
    K_len = n_sites * IF + 1  # +1 = ones column carrying the bias
    full, r = divmod(K_len, P)
    T = full + (1 if r else 0)
    K_pad = T * P  # stationary-side padding only (km2 is tiny)

    # Keff (masked kernel slice) + bias row, zero-padded, fp16, tile-swizzled.
    km = np.zeros((K_pad, F), np.float32)
    if n_sites:
        kr = kernel.reshape(S, IF, S, F)[:n_sites, :, index, :]
        km[: n_sites * IF] = np.asarray(kr, np.float32).reshape(n_sites * IF, F)
    km[n_sites * IF] = np.asarray(bias[index], np.float32)
    khi = km.astype(np.float16)
    KM2 = np.ascontiguousarray(
        khi.reshape(T, P, F).transpose(1, 0, 2).reshape(P, T * F)
    )

    inputs = np.asarray(inputs, np.float32)
    cache = np.asarray(cache, np.float32)
    in_maps = []
    for c in range(NCORES):
        rows = slice(c * B_core, (c + 1) * B_core)
        at = np.zeros((K_len, B_core), np.float32)
        if n_sites:
            at[: n_sites * IF] = (
                cache[rows, :n_sites, :].reshape(B_core, n_sites * IF).T
            )
            at[hi_site * IF : (hi_site + 1) * IF] = inputs[rows].T
        at[n_sites * IF] = 1.0
        ahi = at.astype(np.float16)
        m = {"at2": np.ascontiguousarray(ahi[: full * P]), "km2": KM2}
        if r:
            m["atl"] = np.ascontiguousarray(ahi[full * P :])
        in_maps.append(m)
    return in_maps, K_len, B_core, F


def kernel(inputs, cache, kernel, bias, index):
    from concourse.bass_utils import run_bass_kernel_spmd

    in_maps, K_len, B_core, F = _prepare(inputs, cache, kernel, bias, index)
    nc = _get_nc(K_len, B_core, F)
    res = run_bass_kernel_spmd(nc, in_maps, core_ids=list(range(NCORES)))
    parts = []
    for c in range(NCORES):
        o = np.asarray(res.results[c]["out"])  # (F, B_core) = y^T, fp16
        parts.append(o.T.astype(np.float32))
    return np.ascontiguousarray(np.concatenate(parts, axis=0), dtype=np.float32)


# revision 15
# speedup vs baseline: 3.7303x; 2.2119x over previous
"""Trainium2 Bass kernel for FastMaskedDense1D.update_site (index=300 regime).

Math (reference semantics, EXCLUSIVE=1):
    cache[:, index-1, :] = inputs                      (scatter)
    cache_i = cache[:, :index+1, :].reshape(B, -1)
    kernel_i = kernel.reshape(S, IF, S, F)[:index+1, :, index, :]
    kernel_i *= (arange(index+1) <= index-1)[:, None, None]   (mask)
    y = cache_i @ kernel_i.reshape(-1, F) + bias[index]

Because the mask zeroes site `index`, only sites 0..index-1 contribute, with
site index-1 replaced by `inputs`. So the whole op is one skinny matmul:
    y = A @ Keff + bias,  A: (B, index*IF), Keff: (index*IF, F)

Strategy: data-parallel over the batch across 8 NeuronCores. The host folds
scatter + mask + kernel-slice + bias (ones-column trick) into a per-core
dense problem laid out contraction-major (A^T, contiguous) so each core runs
a single streaming matmul  out^T = Keff^T @ A^T  at full DMA rate.

Precision: the 2e-2 rel-err budget is enormous relative to fp16 rounding
(~4e-4 measured for a 4801-length dot of N(0,1) data), so both A and Keff
ship as plain fp16 — HALF the HBM bytes of fp32 (or of an fp16 hi/lo
pair). fp8 would fail the gate (~3.6e-2) and the PE has no int8, so 2
bytes/element is the true traffic floor (~10.0 MB/core). PSUM accumulates
in fp32 across the 38 k-tiles; one copy casts y^T to fp16 in SBUF at
partition base 0 (walrus-safe), and one 32KB DMA ships it out.

Engine split: the A stream issues from the SP HWDGE queues; kw/tail/out
DMAs ride the Activation HWDGE queues so the main stream never sees
head-of-line blocking at rep/kernel boundaries. PE busy (~9-10us of
512-wide fp16 matmuls) stays under the DMA stream time, keeping the
kernel memory-bound at the HBM roofline (~500-900 GB/s/core depending on
chip contention).

DRAM layout per core:
  at2 (full*128, 1024) fp16 : k-tile-major rows (plain row-major A^T)
  atl (65, 1024) fp16       : the 4801 % 128 tail rows
  km2 (128, T*F) fp16       : per-tile stationary blocks of Keff^T
  out (F, 1024) fp16        : y^T; host transposes + casts back
"""

import ml_dtypes
import numpy as np

BATCH = 8192
SIZE = 512
FEATURES = 16
IN_FEATURES = 16
EXCLUSIVE = 1
NCORES = 8
P = 128
A_FP8 = True  # ship A as fp8 e3m4 (1 B/elt, rel err 1.34e-2); False -> fp16
G = 8 if A_FP8 else 4  # contraction k-tiles per DMA (~1MB per transfer)
SPLIT_B = 1  # DMAs per supertile along the batch dim
BUFS = 6  # deep a-tile pipeline: absorbs HBM-contention jitter

_NC_CACHE: dict = {}


def _build(K_len: int, B: int, F: int, repeats: int = 1):
    """out(F, B) = Keff.T @ A^T via fp16 matmuls.

    K_len need not be a multiple of 128: the first `full` 128-row k-tiles
    stream through the main loop; the remaining r = K_len - full*128 rows
    go through one partial-partition tile (K<128 matmul) so no padded
    bytes ever cross HBM."""
    import concourse.bacc as bacc
    import concourse.mybir as mybir
    from concourse.tile import TileContext

    F32 = mybir.dt.float32
    FP16 = mybir.dt.float16
    ADT = mybir.dt.float8e3 if A_FP8 else FP16
    full, r = divmod(K_len, P)
    T = full + (1 if r else 0)  # stationary blocks in km2
    assert B % 512 == 0
    NBH = B // 512

    nc = bacc.Bacc("TRN2", target_bir_lowering=False, debug=False)
    AT2 = nc.dram_tensor("at2", (full * P, B), ADT, kind="ExternalInput")
    if r:
        ATL = nc.dram_tensor("atl", (r, B), ADT, kind="ExternalInput")
    KM2 = nc.dram_tensor("km2", (P, T * F), FP16, kind="ExternalInput")
    OUT = nc.dram_tensor("out", (F, B), FP16, kind="ExternalOutput")

    at_view = AT2.ap().rearrange("(q p) b -> p q b", p=P)

    with TileContext(nc) as tc:
        with (
            tc.tile_pool(name="kw", bufs=1) as kwpool,
            tc.tile_pool(name="a", bufs=BUFS) as apool,
            tc.tile_pool(name="o", bufs=2) as opool,
            tc.tile_pool(name="ps", bufs=2, space="PSUM") as pspool,
        ):
            # kw/tail/out DMAs ride the Activation HWDGE queues so the SP
            # queue streams A tiles back-to-back with no head-of-line stalls.
            kw = kwpool.tile([P, T * F], FP16)
            nc.scalar.dma_start(kw[:], KM2.ap())
            for _ in range(repeats):
                ps = [
                    pspool.tile([F, 512], F32, tag=f"ps_{bh}", name=f"ps_{bh}")
                    for bh in range(NBH)
                ]
                t = 0
                first = True
                while t < full:
                    # small first transfer so the PE starts ~3 k-tiles
                    # earlier during pipeline fill; full-size after that
                    g = 1 if (first and full > G) else min(G, full - t)
                    first = False
                    a_tile = apool.tile([P, G, B], ADT, tag="a", name="a_tile")
                    bs = B // SPLIT_B
                    for sb in range(SPLIT_B):
                        nc.sync.dma_start(
                            a_tile[:, :g, sb * bs : (sb + 1) * bs],
                            at_view[:, t : t + g, sb * bs : (sb + 1) * bs],
                        )
                    for gi in range(g):
                        tt = t + gi
                        for bh in range(NBH):
                            bsl = slice(bh * 512, (bh + 1) * 512)
                            nc.tensor.matmul(
                                ps[bh][:],
                                kw[:, tt * F : (tt + 1) * F],
                                a_tile[:, gi, bsl],
                                start=(tt == 0),
                                stop=(tt == full - 1 and not r),
                            )
                    t += g
                if r:
                    a_tail = apool.tile([P, 1, B], ADT, tag="atail", name="a_tail")
                    nc.scalar.dma_start(a_tail[:r, 0, :], ATL.ap())
                    for bh in range(NBH):
                        bsl = slice(bh * 512, (bh + 1) * 512)
                        nc.tensor.matmul(
                            ps[bh][:],
                            kw[:r, full * F : (full + 1) * F],
                            a_tail[:r, 0, bsl],
                            start=(full == 0),
                            stop=True,
                        )
                outsb = opool.tile([F, B], FP16, tag="out", name="outsb")
                for bh in range(NBH):
                    bsl = slice(bh * 512, (bh + 1) * 512)
                    nc.any.tensor_copy(out=outsb[:, bsl], in_=ps[bh][:])
                nc.scalar.dma_start(OUT.ap()[:, :], outsb[:, :])
    nc.compile()
    return nc


def _get_nc(K_len: int, B: int, F: int, repeats: int = 1):
    key = (K_len, B, F, repeats)
    if key not in _NC_CACHE:
        _NC_CACHE[key] = _build(K_len, B, F, repeats)
    return _NC_CACHE[key]


def _prepare(inputs, cache, kernel, bias, index):
    """Host-side fold: returns (in_maps, K_len, B_core, F)."""
    index = int(index)
    B, IF = inputs.shape
    S, F = bias.shape
    assert B % NCORES == 0
    B_core = B // NCORES

    hi_site = index - EXCLUSIVE
    n_sites = hi_site + 1 if hi_site >= 0 else 0
    K_len = n_sites * IF + 1  # +1 = ones column carrying the bias
    full, r = divmod(K_len, P)
    T = full + (1 if r else 0)
    K_pad = T * P  # stationary-side padding only (km2 is tiny)

    # Keff (masked kernel slice) + bias row, zero-padded, fp16, tile-swizzled.
    km = np.zeros((K_pad, F), np.float32)
    if n_sites:
        kr = kernel.reshape(S, IF, S, F)[:n_sites, :, index, :]
        km[: n_sites * IF] = np.asarray(kr, np.float32).reshape(n_sites * IF, F)
    km[n_sites * IF] = np.asarray(bias[index], np.float32)
    khi = km.astype(np.float16)
    KM2 = np.ascontiguousarray(
        khi.reshape(T, P, F).transpose(1, 0, 2).reshape(P, T * F)
    )

    inputs = np.asarray(inputs, np.float32)
    cache = np.asarray(cache, np.float32)
    in_maps = []
    for c in range(NCORES):
        rows = slice(c * B_core, (c + 1) * B_core)
        at = np.zeros((K_len, B_core), np.float32)
        if n_sites:
            at[: n_sites * IF] = (
                cache[rows, :n_sites, :].reshape(B_core, n_sites * IF).T
            )
            at[hi_site * IF : (hi_site + 1) * IF] = inputs[rows].T
        at[n_sites * IF] = 1.0
        ahi = at.astype(ml_dtypes.float8_e3m4 if A_FP8 else np.float16)
        m = {"at2": np.ascontiguousarray(ahi[: full * P]), "km2": KM2}
        if r:
            m["atl"] = np.ascontiguousarray(ahi[full * P :])
        in_maps.append(m)
    return in_maps, K_len, B_core, F


def kernel(inputs, cache, kernel, bias, index):
    from concourse.bass_utils import run_bass_kernel_spmd

    in_maps, K_len, B_core, F = _prepare(inputs, cache, kernel, bias, index)
    nc = _get_nc(K_len, B_core, F)
    res = run_bass_kernel_spmd(nc, in_maps, core_ids=list(range(NCORES)))
    parts = []
    for c in range(NCORES):
        o = np.asarray(res.results[c]["out"])  # (F, B_core) = y^T, fp16
        parts.append(o.T.astype(np.float32))
    return np.ascontiguousarray(np.concatenate(parts, axis=0), dtype=np.float32)
